# revision 1
# baseline (speedup 1.0000x reference)
"""Biaffine kernel for Trainium2, data-parallel over batch across 8 NeuronCores.

Math (reference):
  Ha = [H, 1]                                   # [B, N, d+1]
  out[b,x,y] = D[b,x,:] @ U @ Ha[b,y,:]  +  Ha[b,x,:]@W[:d+1]  +  D[b,y,:]@W[d+1:]

Decomposition used here (per batch b):
  U1 = U[:, :d]  (d x d),  u2 = U[:, d]
  G_b  = U1 @ H_b^T                             # [d, N]   (step 1, PE)
  S_b  = D_b @ G_b                              # [N, N]   (step 2, PE)
  rowvec[x] = D_b[x,:]@u2 + H_b[x,:]@W[:d]      # via skinny matmuls (vec3)
  colvec[y] = D_b[y,:]@W[d+1:] + W[d]
  out_b = S_b + rowvec x 1 + 1 x colvec         # folded into step 2 as a K=2 matmul

All matmuls in bf16 with fp32 PSUM accumulation. Host does layout/dtype prep only.
"""

import sys

for _p in ("/opt/trn_rl_repo", "/root/.axon_site/_ro/trn_rl_repo"):
    if _p not in sys.path:
        sys.path.append(_p)

import ml_dtypes
import numpy as np

B, N, DD = 64, 512, 1024
NCORES = 8
BPC = B // NCORES  # batches per core
P = 128
KC = DD // P  # 8 contraction chunks of 128
XC = N // P  # 4 output-row chunks of 128

BF16 = ml_dtypes.bfloat16

LAST_RESULT = None  # BassKernelResults of the most recent run (for test.py)


def _ensure_axon_ntff_hook():
    """Provide antenv.axon_hooks if the image lacks it, so trace=True works
    under axon. No-op when the real module exists or the .so is absent."""
    try:
        import antenv.axon_hooks  # noqa: F401
        return
    except ImportError:
        pass
    import contextlib
    import ctypes
    import os
    import types

    holder = {"hook": None, "built": False}

    def _build_hook():
        so_path = "/opt/axon/libaxon_pjrt.so"
        if not os.path.exists(so_path):
            return None
        lib = ctypes.CDLL(so_path)
        if not hasattr(lib, "axon_start_nrt_profile"):
            return None
        lib.axon_start_nrt_profile.argtypes = [
            ctypes.POINTER(ctypes.c_int64),
            ctypes.c_size_t,
        ]
        lib.axon_start_nrt_profile.restype = ctypes.c_int64
        lib.axon_stop_nrt_profile.argtypes = [ctypes.c_char_p]
        lib.axon_stop_nrt_profile.restype = ctypes.c_int64

        @contextlib.contextmanager
        def _hook(output_dir, device_ids):
            import jax

            jax.devices()
            if device_ids:
                ids = (ctypes.c_int64 * len(device_ids))(*device_ids)
                rc = lib.axon_start_nrt_profile(ids, len(device_ids))
            else:
                rc = lib.axon_start_nrt_profile(None, 0)
            if rc != 0:
                raise RuntimeError(f"axon_start_nrt_profile rc={rc}")
            try:
                yield
            finally:
                n = lib.axon_stop_nrt_profile(str(output_dir).encode())
                print(f"ntff profile: {n} file(s) -> {output_dir}")

        return _hook

    def set_axon_ntff_profile_hook(h):
        holder["hook"] = h
        holder["built"] = True

    def get_axon_ntff_profile_hook():
        if not holder["built"]:
            holder["hook"] = _build_hook()
            holder["built"] = True
        return holder["hook"]

    mod = types.ModuleType("antenv.axon_hooks")
    mod.set_axon_ntff_profile_hook = set_axon_ntff_profile_hook
    mod.get_axon_ntff_profile_hook = get_axon_ntff_profile_hook
    sys.modules["antenv.axon_hooks"] = mod
    try:
        import antenv

        antenv.axon_hooks = mod
    except ImportError:
        pass


def _build_bass(c_const: float):
    import concourse.mybir as mybir
    import concourse.tile as tile
    from concourse import bacc
    from contextlib import ExitStack

    bf = mybir.dt.bfloat16
    f32 = mybir.dt.float32

    nc = bacc.Bacc("TRN2")
    NP = BPC // 2  # batch pairs (ht is loaded/kept as pairs; matmuls stay N=512)
    dtr_h = nc.dram_tensor("dtr", [BPC, P, KC, N], bf, kind="ExternalInput")
    htr_h = nc.dram_tensor("htr", [NP, P, KC, 2 * N], bf, kind="ExternalInput")
    ujt_h = nc.dram_tensor("ujt", [P, KC, DD], bf, kind="ExternalInput")
    vpr_h = nc.dram_tensor("vpr", [P, KC, 2], bf, kind="ExternalInput")
    u2r_h = nc.dram_tensor("u2r", [P, KC], f32, kind="ExternalInput")
    out_h = nc.dram_tensor("out", [BPC, N, N], f32, kind="ExternalOutput")

    with tile.TileContext(nc) as tc, ExitStack() as ctx:
        const_pool = ctx.enter_context(tc.tile_pool(name="const", bufs=1))
        vp_s = const_pool.tile([P, KC, 2], bf, name="vp_s")
        nc.sync.dma_start(vp_s[:], vpr_h[:])
        u2_s = const_pool.tile([P, KC], f32, name="u2_s")
        nc.sync.dma_start(u2_s[:], u2r_h[:])
        ujt_s = const_pool.tile([P, KC, DD], bf, name="ujt_s")

        dpool = ctx.enter_context(tc.tile_pool(name="dpool", bufs=4))
        hpool = ctx.enter_context(tc.tile_pool(name="hpool", bufs=2))
        gpool = ctx.enter_context(tc.tile_pool(name="gpool", bufs=2))
        opool = ctx.enter_context(tc.tile_pool(name="opool", bufs=2))
        vecpool = ctx.enter_context(tc.tile_pool(name="vecpool", bufs=2))
        gps_pool = ctx.enter_context(tc.tile_pool(name="gps", bufs=3, space="PSUM"))
        sps_pool = ctx.enter_context(tc.tile_pool(name="sps", bufs=3, space="PSUM"))
        vps_pool = ctx.enter_context(tc.tile_pool(name="vps", bufs=1, space="PSUM"))

        ones_s = const_pool.tile([1, N], bf, name="ones_s")
        nc.vector.memset(ones_s[:], 1.0)

        state = {}  # per-batch tiles kept alive for the pipelined step 2

        def load_pair(p):
            ht2_t = hpool.tile([P, KC, 2 * N], bf, name=f"ht{p}", tag="ht")
            dts = []
            if p == 0:
                # first half of ht feeds the first matmuls; ujt chunks are only
                # needed after the 32 vec matmuls (~7us of cover)
                nc.sync.dma_start(ht2_t[:, :, 0:N], htr_h[p, :, :, 0:N])
                nc.sync.dma_start(ht2_t[:, :, N:2 * N], htr_h[p, :, :, N:2 * N])
                for b01 in range(2):
                    dt_t = dpool.tile([P, KC, N], bf, name=f"dt{2 * p + b01}", tag="dt")
                    nc.sync.dma_start(dt_t[:], dtr_h[2 * p + b01])
                    dts.append(dt_t)
                for jc in range(KC):
                    nc.sync.dma_start(ujt_s[:, jc, :], ujt_h[:, jc, :])
            else:
                nc.sync.dma_start(ht2_t[:], htr_h[p])
                for b01 in range(2):
                    dt_t = dpool.tile([P, KC, N], bf, name=f"dt{2 * p + b01}", tag="dt")
                    nc.sync.dma_start(dt_t[:], dtr_h[2 * p + b01])
                    dts.append(dt_t)
            return ht2_t, dts

        def vec_mms_pair(p, ht2_t, dts):
            # Four M=1 contraction streams per pair -- rowvec_H = H.wh and
            # colvec = D.wd for both batches -- on distinct 32-col PE groups
            # (tile_position), interleaved per k so they run concurrently.
            # All land in one PSUM bank at quadrant-aligned partitions.
            v4 = vps_pool.tile([97, N], f32, name=f"v4_{p}", tag="v4")
            POS = (0, 32, 64, 96)  # (ra b0, rb b0, ra b1, rb b1)

            def rhs_for(s, k):
                if s == 0:
                    return ht2_t[:, k, 0:N]
                if s == 1:
                    return dts[0][:, k, :]
                if s == 2:
                    return ht2_t[:, k, N:2 * N]
                return dts[1][:, k, :]

            lcol = (0, 1, 0, 1)  # wh for ra streams, wd for rb streams
            for k in range(KC):
                for s in range(4):
                    nc.tensor.matmul(
                        v4[POS[s]:POS[s] + 1, :],
                        lhsT=vp_s[:, k, lcol[s]:lcol[s] + 1],
                        rhs=rhs_for(s, k),
                        start=(k == 0), stop=(k == KC - 1),
                        tile_position=(0, POS[s]),
                        skip_group_check=True,
                    )

            lr = []
            for b01 in range(2):
                b = 2 * p + b01
                # lvec row0 = rowvec_H, row1 = ones ; rvec row0 = ones, row1 = colvec+c
                lvec = vecpool.tile([2, N], bf, name=f"lvec{b}", tag="lvec")
                nc.vector.tensor_copy(lvec[0:1, :], v4[POS[2 * b01]:POS[2 * b01] + 1, :])
                nc.sync.dma_start(lvec[1:2, :], ones_s[0:1, :])
                rvec = vecpool.tile([2, N], bf, name=f"rvec{b}", tag="rvec")
                nc.vector.memset(rvec[0:1, :], 1.0)
                crow = vecpool.tile([1, N], bf, name=f"crow{b}", tag="crow")
                nc.vector.tensor_scalar_add(
                    crow[:], v4[POS[2 * b01 + 1]:POS[2 * b01 + 1] + 1, :], c_const
                )
                nc.sync.dma_start(rvec[1:2, :], crow[0:1, :])
                lr.append((lvec, rvec))
            return lr

        def step1_pair(p, ht2_t, dts):
            lr = vec_mms_pair(p, ht2_t, dts)

            # step 1: G[i, y] = U1 @ H_b^T per batch (matmul free dim caps at 512);
            # the PSUM->SBUF cast adds u2[i] per partition, folding D.u2 into step 2
            g2_t = gpool.tile([P, KC, 2 * N], bf, name=f"g{p}", tag="g")
            for ic in range(KC):
                for b01 in range(2):
                    g_ps = gps_pool.tile([P, N], f32, name=f"gps{p}_{ic}_{b01}", tag="gps")
                    for jc in range(KC):
                        nc.tensor.matmul(
                            g_ps[:],
                            lhsT=ujt_s[:, jc, ic * P:(ic + 1) * P],
                            rhs=ht2_t[:, jc, b01 * N:(b01 + 1) * N],
                            start=(jc == 0), stop=(jc == KC - 1),
                        )
                    nc.vector.tensor_scalar_add(
                        g2_t[:, ic, b01 * N:(b01 + 1) * N], g_ps[:], u2_s[:, ic:ic + 1]
                    )

            for b01 in range(2):
                state[2 * p + b01] = (dts[b01], g2_t, b01, *lr[b01])

        def step2(b):
            dt_t, g2_t, b01, lvec, rvec = state.pop(b)
            o_t = opool.tile([P, XC, N], f32, name=f"o{b}", tag="o")
            for xc in range(XC):
                s_ps = sps_pool.tile([P, N], f32, name=f"sps{b}_{xc}", tag="sps")
                for ic in range(KC):
                    nc.tensor.matmul(
                        s_ps[:],
                        lhsT=dt_t[:, ic, xc * P:(xc + 1) * P],
                        rhs=g2_t[:, ic, b01 * N:(b01 + 1) * N],
                        start=(ic == 0), stop=False,
                    )
                # + rowvec[x] * 1 + 1 * colvec[y]  (one K=2 rank-2 matmul)
                nc.tensor.matmul(
                    s_ps[:],
                    lhsT=lvec[:, xc * P:(xc + 1) * P],
                    rhs=rvec[:, :],
                    start=False, stop=True,
                )
                nc.vector.tensor_copy(o_t[:, xc, :], s_ps[:])
                nc.sync.dma_start(out_h[b, xc * P:(xc + 1) * P, :], o_t[:, xc, :])

        # software-pipelined by one pair so PE never waits on the G copies
        for p in range(NP):
            ht2_t, dts = load_pair(p)
            step1_pair(p, ht2_t, dts)
            if p >= 1:
                step2(2 * p - 2)
                step2(2 * p - 1)
        step2(BPC - 2)
        step2(BPC - 1)

    nc.finalize()
    return nc


def kernel(D, H, U, W, _trace=False):
    global LAST_RESULT
    _ensure_axon_ntff_hook()
    from concourse.bass_utils import run_bass_kernel_spmd

    D = np.asarray(D, dtype=np.float32)
    H = np.asarray(H, dtype=np.float32)
    U = np.asarray(U, dtype=np.float32)
    W = np.asarray(W, dtype=np.float32)

    # ---- host-side layout / dtype prep (no math beyond the W[d] scalar) ----
    # dtr[b, p, c, x] = D[b, x, c*128+p]  (D^T, chunked along the contraction dim)
    DT = D.transpose(0, 2, 1).astype(BF16)  # [B, DD, N]
    dtr = np.ascontiguousarray(DT.reshape(B, KC, P, N).transpose(0, 2, 1, 3))
    HT = H.transpose(0, 2, 1).astype(BF16)
    htr = np.ascontiguousarray(HT.reshape(B, KC, P, N).transpose(0, 2, 1, 3))
    # paired layout for step-1 N=1024 streams: [pair, p, k, (b01, y)]
    htr = np.ascontiguousarray(
        htr.reshape(B // 2, 2, P, KC, N).transpose(0, 2, 3, 1, 4).reshape(B // 2, P, KC, 2 * N)
    )
    # ujt[p, jc, i] = U[i, jc*128+p]
    U1T = U[:, :DD].T  # [j, i]
    ujt = np.ascontiguousarray(U1T.reshape(KC, P, DD).transpose(1, 0, 2)).astype(BF16)
    # vpr[p, c, :] = (wh, wd)[c*128+p] ; u2r[p, c] = u2[c*128+p] (fp32, folded into G)
    vp = np.stack([W[:DD], W[DD + 1:]], axis=1)  # [DD, 2]
    vpr = np.ascontiguousarray(vp.reshape(KC, P, 2).transpose(1, 0, 2)).astype(BF16)
    u2r = np.ascontiguousarray(U[:, DD].reshape(KC, P).T).astype(np.float32)
    c_const = float(W[DD])

    nc = _build_bass(c_const)

    in_maps = []
    for c in range(NCORES):
        sl = slice(c * BPC, (c + 1) * BPC)
        slp = slice(c * (BPC // 2), (c + 1) * (BPC // 2))
        in_maps.append({
            "dtr": dtr[sl],
            "htr": htr[slp],
            "ujt": ujt,
            "vpr": vpr,
            "u2r": u2r,
        })

    try:
        res = run_bass_kernel_spmd(
            nc, in_maps, core_ids=list(range(NCORES)), trace=_trace,
        )
    except Exception:
        # transient device errors (e.g. NRT_EXEC_UNIT_UNRECOVERABLE) usually
        # clear on retry
        res = run_bass_kernel_spmd(
            nc, in_maps, core_ids=list(range(NCORES)), trace=_trace,
        )
    LAST_RESULT = res

    out = np.concatenate([r["out"] for r in res.results], axis=0)
    return np.ascontiguousarray(out.astype(np.float32))


if __name__ == "__main__":
    rng = np.random.default_rng(0)
    D = rng.standard_normal((B, N, DD), dtype=np.float32)
    H = rng.standard_normal((B, N, DD), dtype=np.float32)
    U = (rng.standard_normal((DD, DD + 1)) * 0.02).astype(np.float32)
    W = (rng.standard_normal((2 * DD + 1,)) * 0.02).astype(np.float32)
    out = kernel(D=D, H=H, U=U, W=W)
    print(out.shape, out.dtype)



# revision 12
# speedup vs baseline: 1.0307x; 1.0307x over previous
"""Biaffine kernel for Trainium2, data-parallel over batch across 8 NeuronCores.

Math (reference):
  Ha = [H, 1]                                   # [B, N, d+1]
  out[b,x,y] = D[b,x,:] @ U @ Ha[b,y,:]  +  Ha[b,x,:]@W[:d+1]  +  D[b,y,:]@W[d+1:]

Decomposition used here (per batch b):
  U1 = U[:, :d]  (d x d),  u2 = U[:, d]
  G_b  = U1 @ H_b^T                             # [d, N]   (step 1, PE)
  S_b  = D_b @ G_b                              # [N, N]   (step 2, PE)
  rowvec[x] = D_b[x,:]@u2 + H_b[x,:]@W[:d]      # via skinny matmuls (vec)
  colvec[y] = D_b[y,:]@W[d+1:] + W[d]
  out_b = S_b + rowvec x 1 + 1 x colvec         # folded into step 2 as a K=2 matmul

Schedule: per pair p of batches -- vec(p), step1(p), step2(2p), step2(2p+1).
Pair 0's step1 b01=0 runs jc-major across 8 PSUM banks so the PE consumes
(ujt chunk, ht chunk) pairs in DMA arrival order; warm-up matmuls on a
memset tile cover the DMA lead-in and keep the HAM clock gate hot.
DMA paths: critical pair-0 stream on the Sync HWDGE ring, bulk prefetch on
the GpSimd SWDGE ring, outputs on the Scalar HWDGE ring.
All matmuls in bf16 with fp32 PSUM accumulation.
"""

import sys

for _p in ("/opt/trn_rl_repo", "/root/.axon_site/_ro/trn_rl_repo"):
    if _p not in sys.path:
        sys.path.append(_p)

import ml_dtypes
import numpy as np

B, N, DD = 64, 512, 1024
NCORES = 8
BPC = B // NCORES  # batches per core
P = 128
KC = DD // P  # 8 contraction chunks of 128
XC = N // P  # 4 output-row chunks of 128
NP = BPC // 2  # batch pairs
NWARM = 8  # warm-up matmuls issued before any DMA-dependent work

BF16 = ml_dtypes.bfloat16

LAST_RESULT = None  # BassKernelResults of the most recent run (for test.py)


def _ensure_axon_ntff_hook():
    """Provide antenv.axon_hooks if the image lacks it, so trace=True works
    under axon. No-op when the real module exists or the .so is absent."""
    try:
        import antenv.axon_hooks  # noqa: F401
        return
    except ImportError:
        pass
    import contextlib
    import ctypes
    import os
    import types

    holder = {"hook": None, "built": False}

    def _build_hook():
        so_path = "/opt/axon/libaxon_pjrt.so"
        if not os.path.exists(so_path):
            return None
        lib = ctypes.CDLL(so_path)
        if not hasattr(lib, "axon_start_nrt_profile"):
            return None
        lib.axon_start_nrt_profile.argtypes = [
            ctypes.POINTER(ctypes.c_int64),
            ctypes.c_size_t,
        ]
        lib.axon_start_nrt_profile.restype = ctypes.c_int64
        lib.axon_stop_nrt_profile.argtypes = [ctypes.c_char_p]
        lib.axon_stop_nrt_profile.restype = ctypes.c_int64

        @contextlib.contextmanager
        def _hook(output_dir, device_ids):
            import jax

            jax.devices()
            if device_ids:
                ids = (ctypes.c_int64 * len(device_ids))(*device_ids)
                rc = lib.axon_start_nrt_profile(ids, len(device_ids))
            else:
                rc = lib.axon_start_nrt_profile(None, 0)
            if rc != 0:
                raise RuntimeError(f"axon_start_nrt_profile rc={rc}")
            try:
                yield
            finally:
                n = lib.axon_stop_nrt_profile(str(output_dir).encode())
                print(f"ntff profile: {n} file(s) -> {output_dir}")

        return _hook

    def set_axon_ntff_profile_hook(h):
        holder["hook"] = h
        holder["built"] = True

    def get_axon_ntff_profile_hook():
        if not holder["built"]:
            holder["hook"] = _build_hook()
            holder["built"] = True
        return holder["hook"]

    mod = types.ModuleType("antenv.axon_hooks")
    mod.set_axon_ntff_profile_hook = set_axon_ntff_profile_hook
    mod.get_axon_ntff_profile_hook = get_axon_ntff_profile_hook
    sys.modules["antenv.axon_hooks"] = mod
    try:
        import antenv

        antenv.axon_hooks = mod
    except ImportError:
        pass


def _build_bass(c_const: float):
    import concourse.mybir as mybir
    import concourse.tile as tile
    from concourse import bacc
    from contextlib import ExitStack

    bf = mybir.dt.bfloat16
    f32 = mybir.dt.float32
    ACT = mybir.ActivationFunctionType

    nc = bacc.Bacc("TRN2")
    dtr_h = nc.dram_tensor("dtr", [BPC, P, KC, N], bf, kind="ExternalInput")
    htr_h = nc.dram_tensor("htr", [NP, KC, P, 2 * N], bf, kind="ExternalInput")
    ujt_h = nc.dram_tensor("ujt", [P, KC, DD], bf, kind="ExternalInput")
    vpr_h = nc.dram_tensor("vpr", [P, KC, 2], bf, kind="ExternalInput")
    u2r_h = nc.dram_tensor("u2r", [P, KC], f32, kind="ExternalInput")
    # per-partition (scale, bias) pairs for the lvec/rvec assembly ACT ops:
    # cols = (scale_l, bias_l, scale_r, bias_r)
    cst_h = nc.dram_tensor("cst", [2, 4], f32, kind="ExternalInput")
    out_h = nc.dram_tensor("out", [BPC, N, N], f32, kind="ExternalOutput")

    with tile.TileContext(nc) as tc, ExitStack() as ctx:
        const_pool = ctx.enter_context(tc.tile_pool(name="const", bufs=1))
        vp_s = const_pool.tile([P, KC, 2], bf, name="vp_s")
        u2_s = const_pool.tile([P, KC], f32, name="u2_s")
        ujt_s = const_pool.tile([P, KC, DD], bf, name="ujt_s")
        warm_s = const_pool.tile([P, N], bf, name="warm_s")
        cst_s = const_pool.tile([2, 4], f32, name="cst_s")
        # persistent row/col vector tiles (4-deep: 2 batches/pair x 2 pairs
        # in flight); lvec = [rowH; 1], rvec = [1; colvec+c], both rows
        # written by one 2-partition ACT op per batch
        lvecs = [const_pool.tile([2, N], bf, name=f"lvec{i}") for i in range(4)]
        rvecs = [const_pool.tile([2, N], bf, name=f"rvec{i}") for i in range(4)]

        hpool = ctx.enter_context(tc.tile_pool(name="hpool", bufs=2))
        dpool = ctx.enter_context(tc.tile_pool(name="dpool", bufs=4))
        gpool = ctx.enter_context(tc.tile_pool(name="gpool", bufs=1))
        opool = ctx.enter_context(tc.tile_pool(name="opool", bufs=4))
        pspool = ctx.enter_context(tc.tile_pool(name="ps", bufs=8, space="PSUM"))

        # engine-local init (no DMA deps): warm tile
        nc.gpsimd.memset(warm_s[:], 0.125)

        # warm-up matmuls: cover the DMA lead-in and flip the HAM clock gate
        wps = pspool.tile([P, N], f32, name="warm_ps", tag="ps")
        for i in range(NWARM):
            nc.tensor.matmul(
                wps[:], lhsT=warm_s[:, 0:P], rhs=warm_s[:],
                start=(i == 0), stop=(i == NWARM - 1),
            )

        # ---- critical-path DMA stream (Sync HWDGE ring, consumption order)
        nc.sync.dma_start(vp_s[:], vpr_h[:])
        nc.sync.dma_start(u2_s[:], u2r_h[:])
        nc.sync.dma_start(cst_s[:], cst_h[:])
        ht_tiles = {}
        dt_tiles = {}
        ht0 = hpool.tile([P, KC, 2 * N], bf, name="ht0", tag="ht")
        ht_tiles[0] = ht0
        for jc in range(KC):
            nc.sync.dma_start(ujt_s[:, jc, :], ujt_h[:, jc, :])
            nc.sync.dma_start(ht0[:, jc, 0:N], htr_h[0, jc, :, 0:N])
        for jc in range(KC):
            nc.sync.dma_start(ht0[:, jc, N:2 * N], htr_h[0, jc, :, N:2 * N])

        # ---- bulk prefetch (GpSimd SWDGE ring)
        def load_dt(b):
            dt = dpool.tile([P, KC, N], bf, name=f"dt{b}", tag="dt")
            nc.gpsimd.dma_start(dt[:], dtr_h[b])
            dt_tiles[b] = dt

        def load_ht(p):
            ht = hpool.tile([P, KC, 2 * N], bf, name=f"ht{p}", tag="ht")
            for jc in range(KC):
                nc.gpsimd.dma_start(ht[:, jc, :], htr_h[p, jc])
            ht_tiles[p] = ht

        load_dt(0)
        load_dt(1)
        for p in range(1, NP):
            load_ht(p)
            load_dt(2 * p)
            load_dt(2 * p + 1)

        def vec_pair(p):
            # Four M=2 contraction streams (weights [wh, wd]) on distinct
            # 32-col PE groups, interleaved per k so they run concurrently.
            # ht streams yield rowH at partitions {0, 64} (+junk), dt streams
            # yield colD at partitions {33, 97} (junk at 32/96).
            ht = ht_tiles[p]
            d0, d1 = dt_tiles[2 * p], dt_tiles[2 * p + 1]
            v4 = pspool.tile([98, N], f32, name=f"v4_{p}", tag="ps")
            POS = (0, 32, 64, 96)  # (rowH b0, colD b0, rowH b1, colD b1)

            def rhs_for(s, k):
                if s == 0:
                    return ht[:, k, 0:N]
                if s == 1:
                    return d0[:, k, :]
                if s == 2:
                    return ht[:, k, N:2 * N]
                return d1[:, k, :]

            for k in range(KC):
                for s in range(4):
                    nc.tensor.matmul(
                        v4[POS[s]:POS[s] + 2, :],
                        lhsT=vp_s[:, k, 0:2],
                        rhs=rhs_for(s, k),
                        start=(k == 0), stop=(k == KC - 1),
                        tile_position=(0, POS[s]),
                        skip_group_check=True,
                    )
            # assemble lvec = [rowH; 1] and rvec = [1; colD+c] on the Scalar
            # engine: one 2-partition op each with per-partition scale/bias
            for b01 in range(2):
                b = 2 * p + b01
                lv, rv = lvecs[b % 4], rvecs[b % 4]
                lp, rp = POS[2 * b01], POS[2 * b01 + 1]
                nc.scalar.activation(
                    lv[0:2, :], v4[lp:lp + 2, :], ACT.Identity,
                    scale=cst_s[0:2, 0:1], bias=cst_s[0:2, 1:2],
                )
                nc.scalar.activation(
                    rv[0:2, :], v4[rp:rp + 2, :], ACT.Identity,
                    scale=cst_s[0:2, 2:3], bias=cst_s[0:2, 3:4],
                )

        def step1_pair(p):
            # G[i, y] = U1 @ H_b^T per batch; the PSUM->SBUF copy adds u2[i]
            # per partition, folding D.u2 into step 2.
            ht = ht_tiles[p]
            g2 = gpool.tile([P, KC, 2 * N], bf, name=f"g{p}", tag="g")
            if p == 0:
                # b01=0 jc-major: consumes (ujt[jc], ht[jc]) in DMA order
                banks = [
                    pspool.tile([P, N], f32, name=f"gA{ic}", tag="ps")
                    for ic in range(KC)
                ]
                for jc in range(KC):
                    for ic in range(KC):
                        nc.tensor.matmul(
                            banks[ic][:],
                            lhsT=ujt_s[:, jc, ic * P:(ic + 1) * P],
                            rhs=ht[:, jc, 0:N],
                            start=(jc == 0), stop=(jc == KC - 1),
                        )
                for ic in range(KC):
                    nc.vector.tensor_scalar_add(
                        g2[:, ic, 0:N], banks[ic][:], u2_s[:, ic:ic + 1]
                    )
                b01s = (1,)
            else:
                b01s = (0, 1)
            for ic in range(KC):
                for b01 in b01s:
                    g_ps = pspool.tile([P, N], f32, name=f"gps{p}_{ic}_{b01}", tag="ps")
                    for jc in range(KC):
                        nc.tensor.matmul(
                            g_ps[:],
                            lhsT=ujt_s[:, jc, ic * P:(ic + 1) * P],
                            rhs=ht[:, jc, b01 * N:(b01 + 1) * N],
                            start=(jc == 0), stop=(jc == KC - 1),
                        )
                    nc.vector.tensor_scalar_add(
                        g2[:, ic, b01 * N:(b01 + 1) * N], g_ps[:], u2_s[:, ic:ic + 1]
                    )
            return g2

        def step2(b, g2):
            b01 = b % 2
            dt = dt_tiles[b]
            lv, rv = lvecs[b % 4], rvecs[b % 4]
            for xc in range(XC):
                s_ps = pspool.tile([P, N], f32, name=f"sps{b}_{xc}", tag="ps")
                for ic in range(KC):
                    nc.tensor.matmul(
                        s_ps[:],
                        lhsT=dt[:, ic, xc * P:(xc + 1) * P],
                        rhs=g2[:, ic, b01 * N:(b01 + 1) * N],
                        start=(ic == 0), stop=False,
                    )
                # + rowvec[x] * 1 + 1 * colvec[y]  (one K=2 rank-2 matmul)
                nc.tensor.matmul(
                    s_ps[:],
                    lhsT=lv[:, xc * P:(xc + 1) * P],
                    rhs=rv[:, :],
                    start=False, stop=True,
                )
                o_t = opool.tile([P, N], f32, name=f"o{b}_{xc}", tag="o")
                nc.scalar.activation(o_t[:], s_ps[:], ACT.Copy)
                nc.scalar.dma_start(out_h[b, xc * P:(xc + 1) * P, :], o_t[:])

        g2 = step1_pair(0)
        vec_pair(0)
        step2(0, g2)
        step2(1, g2)
        for p in range(1, NP):
            vec_pair(p)
            g2 = step1_pair(p)
            step2(2 * p, g2)
            step2(2 * p + 1, g2)

    nc.finalize()
    return nc


def kernel(D, H, U, W, _trace=False):
    global LAST_RESULT
    _ensure_axon_ntff_hook()
    from concourse.bass_utils import run_bass_kernel_spmd

    D = np.asarray(D, dtype=np.float32)
    H = np.asarray(H, dtype=np.float32)
    U = np.asarray(U, dtype=np.float32)
    W = np.asarray(W, dtype=np.float32)

    # ---- host-side layout / dtype prep (no math beyond the W[d] scalar) ----
    # dtr[b, p, c, x] = D[b, x, c*128+p]  (D^T, chunked along the contraction dim)
    DT = D.transpose(0, 2, 1).astype(BF16)  # [B, DD, N]
    dtr = np.ascontiguousarray(DT.reshape(B, KC, P, N).transpose(0, 2, 1, 3))
    # htr[pair, jc, p, b01*N+n] = H[2*pair+b01, n, jc*128+p]; each (pair, jc)
    # chunk is one contiguous 512 KB block for big-line DMA
    HT = H.transpose(0, 2, 1).astype(BF16)  # [B, DD, N]
    htr = np.ascontiguousarray(
        HT.reshape(B // 2, 2, KC, P, N).transpose(0, 2, 3, 1, 4)
        .reshape(B // 2, KC, P, 2 * N)
    )
    # ujt[p, jc, i] = U[i, jc*128+p]
    U1T = U[:, :DD].T  # [j, i]
    ujt = np.ascontiguousarray(U1T.reshape(KC, P, DD).transpose(1, 0, 2)).astype(BF16)
    # vpr[p, c, :] = (wh, wd)[c*128+p] ; u2r[p, c] = u2[c*128+p] (fp32, folded into G)
    vp = np.stack([W[:DD], W[DD + 1:]], axis=1)  # [DD, 2]
    vpr = np.ascontiguousarray(vp.reshape(KC, P, 2).transpose(1, 0, 2)).astype(BF16)
    u2r = np.ascontiguousarray(U[:, DD].reshape(KC, P).T).astype(np.float32)
    c_const = float(W[DD])
    # (scale_l, bias_l, scale_r, bias_r) per partition row:
    # lvec row0 = rowH*1+0, row1 = junk*0+1 ; rvec row0 = junk*0+1, row1 = colD*1+c
    cst = np.array(
        [[1.0, 0.0, 0.0, 1.0], [0.0, 1.0, 1.0, c_const]], dtype=np.float32
    )

    nc = _build_bass(c_const)

    in_maps = []
    for c in range(NCORES):
        sl = slice(c * BPC, (c + 1) * BPC)
        slp = slice(c * NP, (c + 1) * NP)
        in_maps.append({
            "dtr": dtr[sl],
            "htr": htr[slp],
            "ujt": ujt,
            "vpr": vpr,
            "u2r": u2r,
            "cst": cst,
        })

    try:
        res = run_bass_kernel_spmd(
            nc, in_maps, core_ids=list(range(NCORES)), trace=_trace,
        )
    except Exception:
        # transient device errors (e.g. NRT_EXEC_UNIT_UNRECOVERABLE) usually
        # clear on retry
        res = run_bass_kernel_spmd(
            nc, in_maps, core_ids=list(range(NCORES)), trace=_trace,
        )
    LAST_RESULT = res

    out = np.concatenate([r["out"] for r in res.results], axis=0)
    return np.ascontiguousarray(out.astype(np.float32))


if __name__ == "__main__":
    rng = np.random.default_rng(0)
    D = rng.standard_normal((B, N, DD), dtype=np.float32)
    H = rng.standard_normal((B, N, DD), dtype=np.float32)
    U = (rng.standard_normal((DD, DD + 1)) * 0.02).astype(np.float32)
    W = (rng.standard_normal((2 * DD + 1,)) * 0.02).astype(np.float32)
    out = kernel(D=D, H=H, U=U, W=W)
    print(out.shape, out.dtype)


# revision 15
# speedup vs baseline: 1.0499x; 1.0186x over previous
"""Biaffine kernel for Trainium2, data-parallel over batch across 8 NeuronCores.

Math (reference):
  Ha = [H, 1]                                   # [B, N, d+1]
  out[b,x,y] = D[b,x,:] @ U @ Ha[b,y,:]  +  Ha[b,x,:]@W[:d+1]  +  D[b,y,:]@W[d+1:]

Decomposition used here (per batch b):
  U1 = U[:, :d]  (d x d),  u2 = U[:, d]
  G_b  = U1 @ H_b^T                             # [d, N]   (step 1, PE)
  S_b  = D_b @ G_b                              # [N, N]   (step 2, PE)
  rowvec[x] = D_b[x,:]@u2 + H_b[x,:]@W[:d]      # via skinny matmuls (vec)
  colvec[y] = D_b[y,:]@W[d+1:] + W[d]
  out_b = S_b + rowvec x 1 + 1 x colvec         # folded into step 2 as a K=2 matmul

Schedule: per pair p of batches -- vec(p), step1(p), step2(2p), step2(2p+1).
Pair 0's step1 b01=0 runs jc-major across 8 PSUM banks so the PE consumes
(ujt chunk, ht chunk) pairs in DMA arrival order; warm-up matmuls on a
memset tile cover the DMA lead-in and keep the HAM clock gate hot.
DMA paths: critical pair-0 stream on the Sync HWDGE ring, bulk prefetch on
the GpSimd SWDGE ring, outputs on the Scalar HWDGE ring.
All matmuls in bf16 with fp32 PSUM accumulation.
"""

import sys

for _p in ("/opt/trn_rl_repo", "/root/.axon_site/_ro/trn_rl_repo"):
    if _p not in sys.path:
        sys.path.append(_p)

import ml_dtypes
import numpy as np

B, N, DD = 64, 512, 1024
NCORES = 8
BPC = B // NCORES  # batches per core
P = 128
KC = DD // P  # 8 contraction chunks of 128
XC = N // P  # 4 output-row chunks of 128
NP = BPC // 2  # batch pairs
NWARM = 12  # warm-up matmuls issued before any DMA-dependent work

BF16 = ml_dtypes.bfloat16

LAST_RESULT = None  # BassKernelResults of the most recent run (for test.py)


def _ensure_axon_ntff_hook():
    """Provide antenv.axon_hooks if the image lacks it, so trace=True works
    under axon. No-op when the real module exists or the .so is absent."""
    try:
        import antenv.axon_hooks  # noqa: F401
        return
    except ImportError:
        pass
    import contextlib
    import ctypes
    import os
    import types

    holder = {"hook": None, "built": False}

    def _build_hook():
        so_path = "/opt/axon/libaxon_pjrt.so"
        if not os.path.exists(so_path):
            return None
        lib = ctypes.CDLL(so_path)
        if not hasattr(lib, "axon_start_nrt_profile"):
            return None
        lib.axon_start_nrt_profile.argtypes = [
            ctypes.POINTER(ctypes.c_int64),
            ctypes.c_size_t,
        ]
        lib.axon_start_nrt_profile.restype = ctypes.c_int64
        lib.axon_stop_nrt_profile.argtypes = [ctypes.c_char_p]
        lib.axon_stop_nrt_profile.restype = ctypes.c_int64

        @contextlib.contextmanager
        def _hook(output_dir, device_ids):
            import jax

            jax.devices()
            if device_ids:
                ids = (ctypes.c_int64 * len(device_ids))(*device_ids)
                rc = lib.axon_start_nrt_profile(ids, len(device_ids))
            else:
                rc = lib.axon_start_nrt_profile(None, 0)
            if rc != 0:
                raise RuntimeError(f"axon_start_nrt_profile rc={rc}")
            try:
                yield
            finally:
                n = lib.axon_stop_nrt_profile(str(output_dir).encode())
                print(f"ntff profile: {n} file(s) -> {output_dir}")

        return _hook

    def set_axon_ntff_profile_hook(h):
        holder["hook"] = h
        holder["built"] = True

    def get_axon_ntff_profile_hook():
        if not holder["built"]:
            holder["hook"] = _build_hook()
            holder["built"] = True
        return holder["hook"]

    mod = types.ModuleType("antenv.axon_hooks")
    mod.set_axon_ntff_profile_hook = set_axon_ntff_profile_hook
    mod.get_axon_ntff_profile_hook = get_axon_ntff_profile_hook
    sys.modules["antenv.axon_hooks"] = mod
    try:
        import antenv

        antenv.axon_hooks = mod
    except ImportError:
        pass


def _build_bass(c_const: float):
    import concourse.mybir as mybir
    import concourse.tile as tile
    from concourse import bacc
    from contextlib import ExitStack

    bf = mybir.dt.bfloat16
    f32 = mybir.dt.float32
    ACT = mybir.ActivationFunctionType

    nc = bacc.Bacc("TRN2")
    dtr_h = nc.dram_tensor("dtr", [BPC, P, KC, N], bf, kind="ExternalInput")
    htr_h = nc.dram_tensor("htr", [NP, KC, P, 2 * N], bf, kind="ExternalInput")
    ujt_h = nc.dram_tensor("ujt", [P, KC, DD], bf, kind="ExternalInput")
    vpr_h = nc.dram_tensor("vpr", [P, KC, 2], bf, kind="ExternalInput")
    u2r_h = nc.dram_tensor("u2r", [P, KC], f32, kind="ExternalInput")
    # per-partition (scale, bias) pairs for the lvec/rvec assembly ACT ops:
    # cols = (scale_l, bias_l, scale_r, bias_r)
    cst_h = nc.dram_tensor("cst", [2, 4], f32, kind="ExternalInput")
    out_h = nc.dram_tensor("out", [BPC, N, N], f32, kind="ExternalOutput")

    with tile.TileContext(nc) as tc, ExitStack() as ctx:
        const_pool = ctx.enter_context(tc.tile_pool(name="const", bufs=1))
        vp_s = const_pool.tile([P, KC, 2], bf, name="vp_s")
        u2_s = const_pool.tile([P, KC], f32, name="u2_s")
        ujt_s = const_pool.tile([P, KC, DD], bf, name="ujt_s")
        warm_s = const_pool.tile([P, N], bf, name="warm_s")
        cst_s = const_pool.tile([2, 4], f32, name="cst_s")
        # persistent row/col vector tiles (4-deep: 2 batches/pair x 2 pairs
        # in flight); lvec = [rowH; 1], rvec = [1; colvec+c], both rows
        # written by one 2-partition ACT op per batch
        lvecs = [const_pool.tile([2, N], bf, name=f"lvec{i}") for i in range(4)]
        rvecs = [const_pool.tile([2, N], bf, name=f"rvec{i}") for i in range(4)]

        hpool = ctx.enter_context(tc.tile_pool(name="hpool", bufs=2))
        dpool = ctx.enter_context(tc.tile_pool(name="dpool", bufs=4))
        gpool = ctx.enter_context(tc.tile_pool(name="gpool", bufs=1))
        opool = ctx.enter_context(tc.tile_pool(name="opool", bufs=4))
        pspool = ctx.enter_context(tc.tile_pool(name="ps", bufs=8, space="PSUM"))

        # engine-local init (no DMA deps): warm tile
        nc.gpsimd.memset(warm_s[:], 0.125)
        gate_s = const_pool.tile([1, 1], bf, name="gate_s")

        # warm-up matmuls (N=128): cover the DMA lead-in at fine granularity
        # and flip the HAM clock gate
        wps = pspool.tile([P, N], f32, name="warm_ps", tag="ps")
        for i in range(NWARM):
            nc.tensor.matmul(
                wps[:, 0:P], lhsT=warm_s[:, 0:P], rhs=warm_s[:, 0:P],
                start=(i == 0), stop=(i == NWARM - 1),
            )

        # ---- critical-path pair-0 DMA, consumption-ordered across both
        # HWDGE rings: ujt/consts/dt on Sync, ht0 on Scalar
        ht_tiles = {}
        dt_tiles = {}
        ht0 = hpool.tile([P, KC, 2 * N], bf, name="ht0", tag="ht")
        ht_tiles[0] = ht0
        for jc in range(KC):
            nc.scalar.dma_start(ht0[:, jc, 0:N], htr_h[0, jc, :, 0:N])
        for jc in range(KC):
            nc.scalar.dma_start(ht0[:, jc, N:2 * N], htr_h[0, jc, :, N:2 * N])
        nc.sync.dma_start(ujt_s[:, 0, :], ujt_h[:, 0, :])
        nc.sync.dma_start(ujt_s[:, 1, :], ujt_h[:, 1, :])
        nc.sync.dma_start(vp_s[:], vpr_h[:])
        nc.sync.dma_start(u2_s[:], u2r_h[:])
        nc.sync.dma_start(cst_s[:], cst_h[:])
        for jc in range(2, KC):
            nc.sync.dma_start(ujt_s[:, jc, :], ujt_h[:, jc, :])

        def load_dt(engine, b):
            dt = dpool.tile([P, KC, N], bf, name=f"dt{b}", tag="dt")
            engine.dma_start(dt[:], dtr_h[b])
            dt_tiles[b] = dt

        load_dt(nc.sync, 0)
        load_dt(nc.sync, 1)

        # ---- bulk prefetch (GpSimd SWDGE ring), gated behind the critical
        # stream so it cannot steal HBM bandwidth from pair 0's loads
        nc.gpsimd.tensor_copy(gate_s[:], dt_tiles[1][0:1, 0:1, 0:1])

        def load_ht(p):
            ht = hpool.tile([P, KC, 2 * N], bf, name=f"ht{p}", tag="ht")
            for jc in range(KC):
                nc.gpsimd.dma_start(ht[:, jc, :], htr_h[p, jc])
            ht_tiles[p] = ht

        for p in range(1, NP):
            load_ht(p)
            load_dt(nc.gpsimd, 2 * p)
            load_dt(nc.gpsimd, 2 * p + 1)

        def vec_pair(p):
            # Four M=2 contraction streams (weights [wh, wd]) on distinct
            # 32-col PE groups, interleaved per k so they run concurrently.
            # ht streams yield rowH at partitions {0, 64} (+junk), dt streams
            # yield colD at partitions {33, 97} (junk at 32/96).
            ht = ht_tiles[p]
            d0, d1 = dt_tiles[2 * p], dt_tiles[2 * p + 1]
            v4 = pspool.tile([98, N], f32, name=f"v4_{p}", tag="ps")
            POS = (0, 32, 64, 96)  # (rowH b0, colD b0, rowH b1, colD b1)

            def rhs_for(s, k):
                if s == 0:
                    return ht[:, k, 0:N]
                if s == 1:
                    return d0[:, k, :]
                if s == 2:
                    return ht[:, k, N:2 * N]
                return d1[:, k, :]

            for k in range(KC):
                for s in range(4):
                    nc.tensor.matmul(
                        v4[POS[s]:POS[s] + 2, :],
                        lhsT=vp_s[:, k, 0:2],
                        rhs=rhs_for(s, k),
                        start=(k == 0), stop=(k == KC - 1),
                        tile_position=(0, POS[s]),
                        skip_group_check=True,
                    )
            # assemble lvec = [rowH; 1] and rvec = [1; colD+c] on the Scalar
            # engine: one 2-partition op each with per-partition scale/bias
            for b01 in range(2):
                b = 2 * p + b01
                lv, rv = lvecs[b % 4], rvecs[b % 4]
                lp, rp = POS[2 * b01], POS[2 * b01 + 1]
                nc.scalar.activation(
                    lv[0:2, :], v4[lp:lp + 2, :], ACT.Identity,
                    scale=cst_s[0:2, 0:1], bias=cst_s[0:2, 1:2],
                )
                nc.scalar.activation(
                    rv[0:2, :], v4[rp:rp + 2, :], ACT.Identity,
                    scale=cst_s[0:2, 2:3], bias=cst_s[0:2, 3:4],
                )

        def step1_pair(p):
            # G[i, y] = U1 @ H_b^T per batch; the PSUM->SBUF copy adds u2[i]
            # per partition, folding D.u2 into step 2.
            ht = ht_tiles[p]
            g2 = gpool.tile([P, KC, 2 * N], bf, name=f"g{p}", tag="g")
            if p == 0:
                # b01=0 jc-major: consumes (ujt[jc], ht[jc]) in DMA order
                banks = [
                    pspool.tile([P, N], f32, name=f"gA{ic}", tag="ps")
                    for ic in range(KC)
                ]
                for jc in range(KC):
                    for ic in range(KC):
                        nc.tensor.matmul(
                            banks[ic][:],
                            lhsT=ujt_s[:, jc, ic * P:(ic + 1) * P],
                            rhs=ht[:, jc, 0:N],
                            start=(jc == 0), stop=(jc == KC - 1),
                        )
                for ic in range(KC):
                    nc.vector.tensor_scalar_add(
                        g2[:, ic, 0:N], banks[ic][:], u2_s[:, ic:ic + 1]
                    )
                b01s = (1,)
            else:
                b01s = (0, 1)
            for ic in range(KC):
                for b01 in b01s:
                    g_ps = pspool.tile([P, N], f32, name=f"gps{p}_{ic}_{b01}", tag="ps")
                    for jc in range(KC):
                        nc.tensor.matmul(
                            g_ps[:],
                            lhsT=ujt_s[:, jc, ic * P:(ic + 1) * P],
                            rhs=ht[:, jc, b01 * N:(b01 + 1) * N],
                            start=(jc == 0), stop=(jc == KC - 1),
                        )
                    nc.vector.tensor_scalar_add(
                        g2[:, ic, b01 * N:(b01 + 1) * N], g_ps[:], u2_s[:, ic:ic + 1]
                    )
            return g2

        def step2(b, g2):
            b01 = b % 2
            dt = dt_tiles[b]
            lv, rv = lvecs[b % 4], rvecs[b % 4]
            for xc in range(XC):
                s_ps = pspool.tile([P, N], f32, name=f"sps{b}_{xc}", tag="ps")
                for ic in range(KC):
                    nc.tensor.matmul(
                        s_ps[:],
                        lhsT=dt[:, ic, xc * P:(xc + 1) * P],
                        rhs=g2[:, ic, b01 * N:(b01 + 1) * N],
                        start=(ic == 0), stop=False,
                    )
                # + rowvec[x] * 1 + 1 * colvec[y]  (one K=2 rank-2 matmul)
                nc.tensor.matmul(
                    s_ps[:],
                    lhsT=lv[:, xc * P:(xc + 1) * P],
                    rhs=rv[:, :],
                    start=False, stop=True,
                )
                o_t = opool.tile([P, N], f32, name=f"o{b}_{xc}", tag="o")
                nc.scalar.activation(o_t[:], s_ps[:], ACT.Copy)
                nc.scalar.dma_start(out_h[b, xc * P:(xc + 1) * P, :], o_t[:])

        g2 = step1_pair(0)
        vec_pair(0)
        step2(0, g2)
        step2(1, g2)
        for p in range(1, NP):
            vec_pair(p)
            g2 = step1_pair(p)
            step2(2 * p, g2)
            step2(2 * p + 1, g2)

    nc.finalize()
    return nc


def kernel(D, H, U, W, _trace=False):
    global LAST_RESULT
    _ensure_axon_ntff_hook()
    from concourse.bass_utils import run_bass_kernel_spmd

    D = np.asarray(D, dtype=np.float32)
    H = np.asarray(H, dtype=np.float32)
    U = np.asarray(U, dtype=np.float32)
    W = np.asarray(W, dtype=np.float32)

    # ---- host-side layout / dtype prep (no math beyond the W[d] scalar) ----
    # dtr[b, p, c, x] = D[b, x, c*128+p]  (D^T, chunked along the contraction dim)
    DT = D.transpose(0, 2, 1).astype(BF16)  # [B, DD, N]
    dtr = np.ascontiguousarray(DT.reshape(B, KC, P, N).transpose(0, 2, 1, 3))
    # htr[pair, jc, p, b01*N+n] = H[2*pair+b01, n, jc*128+p]; each (pair, jc)
    # chunk is one contiguous 512 KB block for big-line DMA
    HT = H.transpose(0, 2, 1).astype(BF16)  # [B, DD, N]
    htr = np.ascontiguousarray(
        HT.reshape(B // 2, 2, KC, P, N).transpose(0, 2, 3, 1, 4)
        .reshape(B // 2, KC, P, 2 * N)
    )
    # ujt[p, jc, i] = U[i, jc*128+p]
    U1T = U[:, :DD].T  # [j, i]
    ujt = np.ascontiguousarray(U1T.reshape(KC, P, DD).transpose(1, 0, 2)).astype(BF16)
    # vpr[p, c, :] = (wh, wd)[c*128+p] ; u2r[p, c] = u2[c*128+p] (fp32, folded into G)
    vp = np.stack([W[:DD], W[DD + 1:]], axis=1)  # [DD, 2]
    vpr = np.ascontiguousarray(vp.reshape(KC, P, 2).transpose(1, 0, 2)).astype(BF16)
    u2r = np.ascontiguousarray(U[:, DD].reshape(KC, P).T).astype(np.float32)
    c_const = float(W[DD])
    # (scale_l, bias_l, scale_r, bias_r) per partition row:
    # lvec row0 = rowH*1+0, row1 = junk*0+1 ; rvec row0 = junk*0+1, row1 = colD*1+c
    cst = np.array(
        [[1.0, 0.0, 0.0, 1.0], [0.0, 1.0, 1.0, c_const]], dtype=np.float32
    )

    nc = _build_bass(c_const)

    in_maps = []
    for c in range(NCORES):
        sl = slice(c * BPC, (c + 1) * BPC)
        slp = slice(c * NP, (c + 1) * NP)
        in_maps.append({
            "dtr": dtr[sl],
            "htr": htr[slp],
            "ujt": ujt,
            "vpr": vpr,
            "u2r": u2r,
            "cst": cst,
        })

    try:
        res = run_bass_kernel_spmd(
            nc, in_maps, core_ids=list(range(NCORES)), trace=_trace,
        )
    except Exception:
        # transient device errors (e.g. NRT_EXEC_UNIT_UNRECOVERABLE) usually
        # clear on retry
        res = run_bass_kernel_spmd(
            nc, in_maps, core_ids=list(range(NCORES)), trace=_trace,
        )
    LAST_RESULT = res

    out = np.concatenate([r["out"] for r in res.results], axis=0)
    return np.ascontiguousarray(out.astype(np.float32))


if __name__ == "__main__":
    rng = np.random.default_rng(0)
    D = rng.standard_normal((B, N, DD), dtype=np.float32)
    H = rng.standard_normal((B, N, DD), dtype=np.float32)
    U = (rng.standard_normal((DD, DD + 1)) * 0.02).astype(np.float32)
    W = (rng.standard_normal((2 * DD + 1,)) * 0.02).astype(np.float32)
    out = kernel(D=D, H=H, U=U, W=W)
    print(out.shape, out.dtype)


# revision 19
# speedup vs baseline: 1.0903x; 1.0385x over previous
"""Biaffine kernel for Trainium2, data-parallel over batch across 8 NeuronCores.

Math (reference):
  Ha = [H, 1]                                   # [B, N, d+1]
  out[b,x,y] = D[b,x,:] @ U @ Ha[b,y,:]  +  Ha[b,x,:]@W[:d+1]  +  D[b,y,:]@W[d+1:]

Decomposition used here (per batch b):
  U1 = U[:, :d]  (d x d),  u2 = U[:, d]
  G_b  = U1 @ H_b^T                             # [d, N]   (step 1, PE)
  S_b  = D_b @ G_b                              # [N, N]   (step 2, PE)
  rowvec[x] = D_b[x,:]@u2 + H_b[x,:]@W[:d]      # via skinny matmuls (vec)
  colvec[y] = D_b[y,:]@W[d+1:] + W[d]
  out_b = S_b + rowvec x 1 + 1 x colvec         # folded into step 2 as a K=2 matmul

Schedule: per pair p of batches -- vec(p), step1(p), step2(2p), step2(2p+1).
Pair 0's step1 b01=0 runs jc-major across 8 PSUM banks so the PE consumes
(ujt chunk, ht chunk) pairs in DMA arrival order; warm-up matmuls on a
memset tile cover the DMA lead-in and keep the HAM clock gate hot.
DMA paths: critical pair-0 stream on the Sync HWDGE ring, bulk prefetch on
the GpSimd SWDGE ring, outputs on the Scalar HWDGE ring.
All matmuls in bf16 with fp32 PSUM accumulation.
"""

import sys

for _p in ("/opt/trn_rl_repo", "/root/.axon_site/_ro/trn_rl_repo"):
    if _p not in sys.path:
        sys.path.append(_p)

import ml_dtypes
import numpy as np

B, N, DD = 64, 512, 1024
NCORES = 8
BPC = B // NCORES  # batches per core
P = 128
KC = DD // P  # 8 contraction chunks of 128
XC = N // P  # 4 output-row chunks of 128
NP = BPC // 2  # batch pairs
NWARM = 20  # warm-up matmuls issued before any DMA-dependent work

BF16 = ml_dtypes.bfloat16

LAST_RESULT = None  # BassKernelResults of the most recent run (for test.py)


def _ensure_axon_ntff_hook():
    """Provide antenv.axon_hooks if the image lacks it, so trace=True works
    under axon. No-op when the real module exists or the .so is absent."""
    try:
        import antenv.axon_hooks  # noqa: F401
        return
    except ImportError:
        pass
    import contextlib
    import ctypes
    import os
    import types

    holder = {"hook": None, "built": False}

    def _build_hook():
        so_path = "/opt/axon/libaxon_pjrt.so"
        if not os.path.exists(so_path):
            return None
        lib = ctypes.CDLL(so_path)
        if not hasattr(lib, "axon_start_nrt_profile"):
            return None
        lib.axon_start_nrt_profile.argtypes = [
            ctypes.POINTER(ctypes.c_int64),
            ctypes.c_size_t,
        ]
        lib.axon_start_nrt_profile.restype = ctypes.c_int64
        lib.axon_stop_nrt_profile.argtypes = [ctypes.c_char_p]
        lib.axon_stop_nrt_profile.restype = ctypes.c_int64

        @contextlib.contextmanager
        def _hook(output_dir, device_ids):
            import jax

            jax.devices()
            if device_ids:
                ids = (ctypes.c_int64 * len(device_ids))(*device_ids)
                rc = lib.axon_start_nrt_profile(ids, len(device_ids))
            else:
                rc = lib.axon_start_nrt_profile(None, 0)
            if rc != 0:
                raise RuntimeError(f"axon_start_nrt_profile rc={rc}")
            try:
                yield
            finally:
                n = lib.axon_stop_nrt_profile(str(output_dir).encode())
                print(f"ntff profile: {n} file(s) -> {output_dir}")

        return _hook

    def set_axon_ntff_profile_hook(h):
        holder["hook"] = h
        holder["built"] = True

    def get_axon_ntff_profile_hook():
        if not holder["built"]:
            holder["hook"] = _build_hook()
            holder["built"] = True
        return holder["hook"]

    mod = types.ModuleType("antenv.axon_hooks")
    mod.set_axon_ntff_profile_hook = set_axon_ntff_profile_hook
    mod.get_axon_ntff_profile_hook = get_axon_ntff_profile_hook
    sys.modules["antenv.axon_hooks"] = mod
    try:
        import antenv

        antenv.axon_hooks = mod
    except ImportError:
        pass


def _build_bass(c_const: float):
    import concourse.mybir as mybir
    import concourse.tile as tile
    from concourse import bacc
    from contextlib import ExitStack

    bf = mybir.dt.bfloat16
    f32 = mybir.dt.float32
    ACT = mybir.ActivationFunctionType

    nc = bacc.Bacc("TRN2")
    dtr_h = nc.dram_tensor("dtr", [BPC, P, KC, N], bf, kind="ExternalInput")
    htr_h = nc.dram_tensor("htr", [NP, KC, P, 2 * N], bf, kind="ExternalInput")
    ujt_h = nc.dram_tensor("ujt", [P, KC, DD], bf, kind="ExternalInput")
    vpr_h = nc.dram_tensor("vpr", [P, KC, 2], bf, kind="ExternalInput")
    u2r_h = nc.dram_tensor("u2r", [P, KC], f32, kind="ExternalInput")
    # per-partition (scale, bias) pairs for the lvec/rvec assembly ACT ops:
    # cols = (scale_l, bias_l, scale_r, bias_r)
    cst_h = nc.dram_tensor("cst", [2, 4], f32, kind="ExternalInput")
    out_h = nc.dram_tensor("out", [BPC, N, N], f32, kind="ExternalOutput")

    with tile.TileContext(nc) as tc, ExitStack() as ctx:
        const_pool = ctx.enter_context(tc.tile_pool(name="const", bufs=1))
        vp_s = const_pool.tile([P, KC, 2], bf, name="vp_s")
        u2_s = const_pool.tile([P, KC], f32, name="u2_s")
        ujt_s = const_pool.tile([P, KC, DD], bf, name="ujt_s")
        warm_s = const_pool.tile([P, N], bf, name="warm_s")
        cst_s = const_pool.tile([2, 4], f32, name="cst_s")
        # persistent row/col vector tiles (4-deep: 2 batches/pair x 2 pairs
        # in flight); lvec = [rowH; 1], rvec = [1; colvec+c], both rows
        # written by one 2-partition ACT op per batch
        lvecs = [const_pool.tile([2, N], bf, name=f"lvec{i}") for i in range(4)]
        rvecs = [const_pool.tile([2, N], bf, name=f"rvec{i}") for i in range(4)]

        hpool = ctx.enter_context(tc.tile_pool(name="hpool", bufs=2))
        dpool = ctx.enter_context(tc.tile_pool(name="dpool", bufs=4))
        gpool = ctx.enter_context(tc.tile_pool(name="gpool", bufs=1))
        opool = ctx.enter_context(tc.tile_pool(name="opool", bufs=4))
        pspool = ctx.enter_context(tc.tile_pool(name="ps", bufs=8, space="PSUM"))

        # engine-local init (no DMA deps): warm tile
        nc.gpsimd.memset(warm_s[:], 0.125)
        gate_s = const_pool.tile([1, 1], bf, name="gate_s")

        # warm-up matmuls (N=128): cover the DMA lead-in at fine granularity
        # and flip the HAM clock gate
        wps = pspool.tile([P, N], f32, name="warm_ps", tag="ps")
        for i in range(NWARM):
            nc.tensor.matmul(
                wps[:, 0:P], lhsT=warm_s[:, 0:P], rhs=warm_s[:, 0:P],
                start=(i == 0), stop=(i == NWARM - 1),
            )

        # ---- critical-path pair-0 DMA in big consumption-ordered chunks
        # across both HWDGE rings: ujt/consts/dt on Sync, ht0 on Scalar
        ht_tiles = {}
        dt_tiles = {}
        ht0 = hpool.tile([P, KC, 2 * N], bf, name="ht0", tag="ht")
        ht_tiles[0] = ht0
        for jc0 in range(0, KC, 4):
            nc.scalar.dma_start(
                ht0[:, jc0:jc0 + 4, 0:N],
                htr_h[0, jc0:jc0 + 4, :, 0:N].rearrange("j p n -> p j n"),
            )
        for jc0 in range(0, KC, 4):
            nc.scalar.dma_start(
                ht0[:, jc0:jc0 + 4, N:2 * N],
                htr_h[0, jc0:jc0 + 4, :, N:2 * N].rearrange("j p n -> p j n"),
            )
        for jc0 in range(0, KC, 2):
            nc.sync.dma_start(ujt_s[:, jc0:jc0 + 2, :], ujt_h[:, jc0:jc0 + 2, :])
        nc.sync.dma_start(u2_s[:], u2r_h[:])
        nc.sync.dma_start(vp_s[:], vpr_h[:])
        nc.sync.dma_start(cst_s[:], cst_h[:])

        def load_dt(engine, b, gate=None):
            dt = dpool.tile([P, KC, N], bf, name=f"dt{b}", tag="dt")
            if gate is not None:
                nc.gpsimd.tensor_copy(dt[0:1, 0, 0:1], gate)
            engine.dma_start(dt[:], dtr_h[b])
            dt_tiles[b] = dt

        load_dt(nc.sync, 0)
        load_dt(nc.sync, 1)

        # ---- bulk prefetch (GpSimd SWDGE ring). Each transfer is gated
        # behind pair 0's critical stream via a 1-element corner write that
        # depends on dt1 -- a WAW dependency the scheduler cannot hoist.
        gate = dt_tiles[1][0:1, 0, 0:1]

        def load_ht(p):
            ht = hpool.tile([P, KC, 2 * N], bf, name=f"ht{p}", tag="ht")
            for jc0 in range(0, KC, 4):
                nc.gpsimd.tensor_copy(ht[0:1, jc0, 0:1], gate)
                nc.gpsimd.dma_start(
                    ht[:, jc0:jc0 + 4, :],
                    htr_h[p, jc0:jc0 + 4].rearrange("j p n -> p j n"),
                )
            ht_tiles[p] = ht

        for p in range(1, NP):
            load_ht(p)
            load_dt(nc.gpsimd, 2 * p, gate=gate)
            load_dt(nc.gpsimd, 2 * p + 1, gate=gate)

        def vec_pair(p):
            # Four M=2 contraction streams (weights [wh, wd]) on distinct
            # 32-col PE groups, interleaved per k so they run concurrently.
            # ht streams yield rowH at partitions {0, 64} (+junk), dt streams
            # yield colD at partitions {33, 97} (junk at 32/96).
            ht = ht_tiles[p]
            d0, d1 = dt_tiles[2 * p], dt_tiles[2 * p + 1]
            v4 = pspool.tile([98, N], f32, name=f"v4_{p}", tag="ps")
            POS = (0, 32, 64, 96)  # (rowH b0, colD b0, rowH b1, colD b1)

            def rhs_for(s, k):
                if s == 0:
                    return ht[:, k, 0:N]
                if s == 1:
                    return d0[:, k, :]
                if s == 2:
                    return ht[:, k, N:2 * N]
                return d1[:, k, :]

            for k in range(KC):
                for s in range(4):
                    nc.tensor.matmul(
                        v4[POS[s]:POS[s] + 2, :],
                        lhsT=vp_s[:, k, 0:2],
                        rhs=rhs_for(s, k),
                        start=(k == 0), stop=(k == KC - 1),
                        tile_position=(0, POS[s]),
                        skip_group_check=True,
                    )
            # assemble lvec = [rowH; 1] and rvec = [1; colD+c] on the Scalar
            # engine: one 2-partition op each with per-partition scale/bias
            for b01 in range(2):
                b = 2 * p + b01
                lv, rv = lvecs[b % 4], rvecs[b % 4]
                lp, rp = POS[2 * b01], POS[2 * b01 + 1]
                nc.scalar.activation(
                    lv[0:2, :], v4[lp:lp + 2, :], ACT.Identity,
                    scale=cst_s[0:2, 0:1], bias=cst_s[0:2, 1:2],
                )
                nc.scalar.activation(
                    rv[0:2, :], v4[rp:rp + 2, :], ACT.Identity,
                    scale=cst_s[0:2, 2:3], bias=cst_s[0:2, 3:4],
                )

        def step1_pair(p):
            # G[i, y] = U1 @ H_b^T per batch; the PSUM->SBUF copy adds u2[i]
            # per partition, folding D.u2 into step 2.
            ht = ht_tiles[p]
            g2 = gpool.tile([P, KC, 2 * N], bf, name=f"g{p}", tag="g")
            if p == 0:
                # b01=0 jc-major: consumes (ujt[jc], ht[jc]) in DMA order
                banks = [
                    pspool.tile([P, N], f32, name=f"gA{ic}", tag="ps")
                    for ic in range(KC)
                ]
                for jc in range(KC):
                    for ic in range(KC):
                        nc.tensor.matmul(
                            banks[ic][:],
                            lhsT=ujt_s[:, jc, ic * P:(ic + 1) * P],
                            rhs=ht[:, jc, 0:N],
                            start=(jc == 0), stop=(jc == KC - 1),
                        )
                for ic in range(KC):
                    nc.vector.tensor_scalar_add(
                        g2[:, ic, 0:N], banks[ic][:], u2_s[:, ic:ic + 1]
                    )
                b01s = (1,)
            else:
                b01s = (0, 1)
            for ic in range(KC):
                for b01 in b01s:
                    g_ps = pspool.tile([P, N], f32, name=f"gps{p}_{ic}_{b01}", tag="ps")
                    for jc in range(KC):
                        nc.tensor.matmul(
                            g_ps[:],
                            lhsT=ujt_s[:, jc, ic * P:(ic + 1) * P],
                            rhs=ht[:, jc, b01 * N:(b01 + 1) * N],
                            start=(jc == 0), stop=(jc == KC - 1),
                        )
                    nc.vector.tensor_scalar_add(
                        g2[:, ic, b01 * N:(b01 + 1) * N], g_ps[:], u2_s[:, ic:ic + 1]
                    )
            return g2

        def step2(b, g2):
            b01 = b % 2
            dt = dt_tiles[b]
            lv, rv = lvecs[b % 4], rvecs[b % 4]
            for xc in range(XC):
                s_ps = pspool.tile([P, N], f32, name=f"sps{b}_{xc}", tag="ps")
                for ic in range(KC):
                    nc.tensor.matmul(
                        s_ps[:],
                        lhsT=dt[:, ic, xc * P:(xc + 1) * P],
                        rhs=g2[:, ic, b01 * N:(b01 + 1) * N],
                        start=(ic == 0), stop=False,
                    )
                # + rowvec[x] * 1 + 1 * colvec[y]  (one K=2 rank-2 matmul)
                nc.tensor.matmul(
                    s_ps[:],
                    lhsT=lv[:, xc * P:(xc + 1) * P],
                    rhs=rv[:, :],
                    start=False, stop=True,
                )
                o_t = opool.tile([P, N], f32, name=f"o{b}_{xc}", tag="o")
                nc.scalar.activation(o_t[:], s_ps[:], ACT.Copy)
                nc.scalar.dma_start(out_h[b, xc * P:(xc + 1) * P, :], o_t[:])

        g2 = step1_pair(0)
        vec_pair(0)
        step2(0, g2)
        step2(1, g2)
        for p in range(1, NP):
            vec_pair(p)
            g2 = step1_pair(p)
            step2(2 * p, g2)
            step2(2 * p + 1, g2)

    nc.finalize()
    return nc


def kernel(D, H, U, W, _trace=False):
    global LAST_RESULT
    _ensure_axon_ntff_hook()
    from concourse.bass_utils import run_bass_kernel_spmd

    D = np.asarray(D, dtype=np.float32)
    H = np.asarray(H, dtype=np.float32)
    U = np.asarray(U, dtype=np.float32)
    W = np.asarray(W, dtype=np.float32)

    # ---- host-side layout / dtype prep (no math beyond the W[d] scalar) ----
    # dtr[b, p, c, x] = D[b, x, c*128+p]  (D^T, chunked along the contraction dim)
    DT = D.transpose(0, 2, 1).astype(BF16)  # [B, DD, N]
    dtr = np.ascontiguousarray(DT.reshape(B, KC, P, N).transpose(0, 2, 1, 3))
    # htr[pair, jc, p, b01*N+n] = H[2*pair+b01, n, jc*128+p]; each (pair, jc)
    # chunk is one contiguous 512 KB block for big-line DMA
    HT = H.transpose(0, 2, 1).astype(BF16)  # [B, DD, N]
    htr = np.ascontiguousarray(
        HT.reshape(B // 2, 2, KC, P, N).transpose(0, 2, 3, 1, 4)
        .reshape(B // 2, KC, P, 2 * N)
    )
    # ujt[p, jc, i] = U[i, jc*128+p]
    U1T = U[:, :DD].T  # [j, i]
    ujt = np.ascontiguousarray(U1T.reshape(KC, P, DD).transpose(1, 0, 2)).astype(BF16)
    # vpr[p, c, :] = (wh, wd)[c*128+p] ; u2r[p, c] = u2[c*128+p] (fp32, folded into G)
    vp = np.stack([W[:DD], W[DD + 1:]], axis=1)  # [DD, 2]
    vpr = np.ascontiguousarray(vp.reshape(KC, P, 2).transpose(1, 0, 2)).astype(BF16)
    u2r = np.ascontiguousarray(U[:, DD].reshape(KC, P).T).astype(np.float32)
    c_const = float(W[DD])
    # (scale_l, bias_l, scale_r, bias_r) per partition row:
    # lvec row0 = rowH*1+0, row1 = junk*0+1 ; rvec row0 = junk*0+1, row1 = colD*1+c
    cst = np.array(
        [[1.0, 0.0, 0.0, 1.0], [0.0, 1.0, 1.0, c_const]], dtype=np.float32
    )

    nc = _build_bass(c_const)

    in_maps = []
    for c in range(NCORES):
        sl = slice(c * BPC, (c + 1) * BPC)
        slp = slice(c * NP, (c + 1) * NP)
        in_maps.append({
            "dtr": dtr[sl],
            "htr": htr[slp],
            "ujt": ujt,
            "vpr": vpr,
            "u2r": u2r,
            "cst": cst,
        })

    try:
        res = run_bass_kernel_spmd(
            nc, in_maps, core_ids=list(range(NCORES)), trace=_trace,
        )
    except Exception:
        # transient device errors (e.g. NRT_EXEC_UNIT_UNRECOVERABLE) usually
        # clear on retry
        res = run_bass_kernel_spmd(
            nc, in_maps, core_ids=list(range(NCORES)), trace=_trace,
        )
    LAST_RESULT = res

    out = np.concatenate([r["out"] for r in res.results], axis=0)
    return np.ascontiguousarray(out.astype(np.float32))


if __name__ == "__main__":
    rng = np.random.default_rng(0)
    D = rng.standard_normal((B, N, DD), dtype=np.float32)
    H = rng.standard_normal((B, N, DD), dtype=np.float32)
    U = (rng.standard_normal((DD, DD + 1)) * 0.02).astype(np.float32)
    W = (rng.standard_normal((2 * DD + 1,)) * 0.02).astype(np.float32)
    out = kernel(D=D, H=H, U=U, W=W)
    print(out.shape, out.dtype)


# revision 21
# speedup vs baseline: 1.1171x; 1.0246x over previous
"""Biaffine kernel for Trainium2, data-parallel over batch across 8 NeuronCores.

Math (reference):
  Ha = [H, 1]                                   # [B, N, d+1]
  out[b,x,y] = D[b,x,:] @ U @ Ha[b,y,:]  +  Ha[b,x,:]@W[:d+1]  +  D[b,y,:]@W[d+1:]

Decomposition used here (per batch b):
  U1 = U[:, :d]  (d x d),  u2 = U[:, d]
  G_b  = U1 @ H_b^T                             # [d, N]   (step 1, PE)
  S_b  = D_b @ G_b                              # [N, N]   (step 2, PE)
  rowvec[x] = D_b[x,:]@u2 + H_b[x,:]@W[:d]      # via skinny matmuls (vec)
  colvec[y] = D_b[y,:]@W[d+1:] + W[d]
  out_b = S_b + rowvec x 1 + 1 x colvec         # folded into step 2 as a K=2 matmul

Schedule: per pair p of batches -- vec(p), step1(p), step2(2p), step2(2p+1).
Pair 0's step1 b01=0 runs jc-major across 8 PSUM banks so the PE consumes
(ujt chunk, ht chunk) pairs in DMA arrival order; warm-up matmuls on a
memset tile cover the DMA lead-in and keep the HAM clock gate hot.
DMA paths: critical pair-0 stream on the Sync HWDGE ring, bulk prefetch on
the GpSimd SWDGE ring, outputs on the Scalar HWDGE ring.
All matmuls in bf16 with fp32 PSUM accumulation.
"""

import sys

for _p in ("/opt/trn_rl_repo", "/root/.axon_site/_ro/trn_rl_repo"):
    if _p not in sys.path:
        sys.path.append(_p)

import ml_dtypes
import numpy as np

B, N, DD = 64, 512, 1024
NCORES = 8
BPC = B // NCORES  # batches per core
P = 128
KC = DD // P  # 8 contraction chunks of 128
XC = N // P  # 4 output-row chunks of 128
NP = BPC // 2  # batch pairs
NWARM = 42  # warm-up matmuls issued before any DMA-dependent work

BF16 = ml_dtypes.bfloat16

LAST_RESULT = None  # BassKernelResults of the most recent run (for test.py)


def _ensure_axon_ntff_hook():
    """Provide antenv.axon_hooks if the image lacks it, so trace=True works
    under axon. No-op when the real module exists or the .so is absent."""
    try:
        import antenv.axon_hooks  # noqa: F401
        return
    except ImportError:
        pass
    import contextlib
    import ctypes
    import os
    import types

    holder = {"hook": None, "built": False}

    def _build_hook():
        so_path = "/opt/axon/libaxon_pjrt.so"
        if not os.path.exists(so_path):
            return None
        lib = ctypes.CDLL(so_path)
        if not hasattr(lib, "axon_start_nrt_profile"):
            return None
        lib.axon_start_nrt_profile.argtypes = [
            ctypes.POINTER(ctypes.c_int64),
            ctypes.c_size_t,
        ]
        lib.axon_start_nrt_profile.restype = ctypes.c_int64
        lib.axon_stop_nrt_profile.argtypes = [ctypes.c_char_p]
        lib.axon_stop_nrt_profile.restype = ctypes.c_int64

        @contextlib.contextmanager
        def _hook(output_dir, device_ids):
            import jax

            jax.devices()
            if device_ids:
                ids = (ctypes.c_int64 * len(device_ids))(*device_ids)
                rc = lib.axon_start_nrt_profile(ids, len(device_ids))
            else:
                rc = lib.axon_start_nrt_profile(None, 0)
            if rc != 0:
                raise RuntimeError(f"axon_start_nrt_profile rc={rc}")
            try:
                yield
            finally:
                n = lib.axon_stop_nrt_profile(str(output_dir).encode())
                print(f"ntff profile: {n} file(s) -> {output_dir}")

        return _hook

    def set_axon_ntff_profile_hook(h):
        holder["hook"] = h
        holder["built"] = True

    def get_axon_ntff_profile_hook():
        if not holder["built"]:
            holder["hook"] = _build_hook()
            holder["built"] = True
        return holder["hook"]

    mod = types.ModuleType("antenv.axon_hooks")
    mod.set_axon_ntff_profile_hook = set_axon_ntff_profile_hook
    mod.get_axon_ntff_profile_hook = get_axon_ntff_profile_hook
    sys.modules["antenv.axon_hooks"] = mod
    try:
        import antenv

        antenv.axon_hooks = mod
    except ImportError:
        pass


def _build_bass(c_const: float):
    import concourse.mybir as mybir
    import concourse.tile as tile
    from concourse import bacc
    from contextlib import ExitStack

    bf = mybir.dt.bfloat16
    f32 = mybir.dt.float32
    ACT = mybir.ActivationFunctionType

    nc = bacc.Bacc("TRN2")
    dtr_h = nc.dram_tensor("dtr", [BPC, P, KC, N], bf, kind="ExternalInput")
    htr_h = nc.dram_tensor("htr", [NP, KC, P, 2 * N], bf, kind="ExternalInput")
    ujt_h = nc.dram_tensor("ujt", [P, KC, DD], bf, kind="ExternalInput")
    vpr_h = nc.dram_tensor("vpr", [P, KC, 2], bf, kind="ExternalInput")
    u2r_h = nc.dram_tensor("u2r", [P, KC], f32, kind="ExternalInput")
    # per-partition (scale, bias) pairs for the lvec/rvec assembly ACT ops:
    # cols = (scale_l, bias_l, scale_r, bias_r)
    cst_h = nc.dram_tensor("cst", [2, 4], f32, kind="ExternalInput")
    out_h = nc.dram_tensor("out", [BPC, N, N], f32, kind="ExternalOutput")

    with tile.TileContext(nc) as tc, ExitStack() as ctx:
        const_pool = ctx.enter_context(tc.tile_pool(name="const", bufs=1))
        vp_s = const_pool.tile([P, KC, 2], bf, name="vp_s")
        u2_s = const_pool.tile([P, KC], f32, name="u2_s")
        ujt_s = const_pool.tile([P, KC, DD], bf, name="ujt_s")
        warm_s = const_pool.tile([P, N], bf, name="warm_s")
        cst_s = const_pool.tile([2, 4], f32, name="cst_s")
        # persistent row/col vector tiles (4-deep: 2 batches/pair x 2 pairs
        # in flight); lvec = [rowH; 1], rvec = [1; colvec+c], both rows
        # written by one 2-partition ACT op per batch
        lvecs = [const_pool.tile([2, N], bf, name=f"lvec{i}") for i in range(4)]
        rvecs = [const_pool.tile([2, N], bf, name=f"rvec{i}") for i in range(4)]

        hpool = ctx.enter_context(tc.tile_pool(name="hpool", bufs=2))
        dpool = ctx.enter_context(tc.tile_pool(name="dpool", bufs=4))
        gpool = ctx.enter_context(tc.tile_pool(name="gpool", bufs=1))
        opool = ctx.enter_context(tc.tile_pool(name="opool", bufs=4))
        pspool = ctx.enter_context(tc.tile_pool(name="ps", bufs=8, space="PSUM"))

        # engine-local init (no DMA deps): warm tile
        nc.gpsimd.memset(warm_s[:], 0.125)
        gate_s = const_pool.tile([1, 1], bf, name="gate_s")

        # warm-up matmuls (N=128): cover the DMA lead-in at fine granularity
        # and flip the HAM clock gate
        wps = pspool.tile([P, N], f32, name="warm_ps", tag="ps")
        for i in range(NWARM):
            nc.tensor.matmul(
                wps[:, 0:P], lhsT=warm_s[:, 0:P], rhs=warm_s[:, 0:P],
                start=(i == 0), stop=(i == NWARM - 1),
            )

        # ---- critical-path pair-0 DMA in big consumption-ordered chunks
        # across both HWDGE rings: ujt/consts/dt on Sync, ht0 on Scalar
        ht_tiles = {}
        dt_tiles = {}
        ht0 = hpool.tile([P, KC, 2 * N], bf, name="ht0", tag="ht")
        ht_tiles[0] = ht0
        for jc0 in range(0, KC, 4):
            nc.scalar.dma_start(
                ht0[:, jc0:jc0 + 4, 0:N],
                htr_h[0, jc0:jc0 + 4, :, 0:N].rearrange("j p n -> p j n"),
            )
        for jc0 in range(0, KC, 4):
            nc.scalar.dma_start(
                ht0[:, jc0:jc0 + 4, N:2 * N],
                htr_h[0, jc0:jc0 + 4, :, N:2 * N].rearrange("j p n -> p j n"),
            )
        for jc0 in range(0, KC, 2):
            nc.sync.dma_start(ujt_s[:, jc0:jc0 + 2, :], ujt_h[:, jc0:jc0 + 2, :])
        nc.sync.dma_start(u2_s[:], u2r_h[:])
        nc.sync.dma_start(vp_s[:], vpr_h[:])
        nc.sync.dma_start(cst_s[:], cst_h[:])

        def load_dt(engine, b, gate=None):
            dt = dpool.tile([P, KC, N], bf, name=f"dt{b}", tag="dt")
            if gate is not None:
                nc.gpsimd.tensor_copy(dt[0:1, 0, 0:1], gate)
            engine.dma_start(dt[:], dtr_h[b])
            dt_tiles[b] = dt

        load_dt(nc.sync, 0)
        load_dt(nc.sync, 1)

        # ---- bulk prefetch (GpSimd SWDGE ring). Each transfer is gated
        # behind pair 0's critical stream via a 1-element corner write that
        # depends on dt1 -- a WAW dependency the scheduler cannot hoist.
        gate = dt_tiles[1][0:1, 0, 0:1]

        def load_ht(p):
            ht = hpool.tile([P, KC, 2 * N], bf, name=f"ht{p}", tag="ht")
            for jc0 in range(0, KC, 4):
                nc.gpsimd.tensor_copy(ht[0:1, jc0, 0:1], gate)
                nc.gpsimd.dma_start(
                    ht[:, jc0:jc0 + 4, :],
                    htr_h[p, jc0:jc0 + 4].rearrange("j p n -> p j n"),
                )
            ht_tiles[p] = ht

        for p in range(1, NP):
            load_ht(p)
            load_dt(nc.gpsimd, 2 * p, gate=gate)
            load_dt(nc.gpsimd, 2 * p + 1, gate=gate)

        def vec_pair(p):
            # Four M=2 contraction streams (weights [wh, wd]) on distinct
            # 32-col PE groups, interleaved per k so they run concurrently.
            # ht streams yield rowH at partitions {0, 64} (+junk), dt streams
            # yield colD at partitions {33, 97} (junk at 32/96).
            ht = ht_tiles[p]
            d0, d1 = dt_tiles[2 * p], dt_tiles[2 * p + 1]
            v4 = pspool.tile([98, N], f32, name=f"v4_{p}", tag="ps")
            POS = (0, 32, 64, 96)  # (rowH b0, colD b0, rowH b1, colD b1)

            def rhs_for(s, k):
                if s == 0:
                    return ht[:, k, 0:N]
                if s == 1:
                    return d0[:, k, :]
                if s == 2:
                    return ht[:, k, N:2 * N]
                return d1[:, k, :]

            for k in range(KC):
                for s in range(4):
                    nc.tensor.matmul(
                        v4[POS[s]:POS[s] + 2, :],
                        lhsT=vp_s[:, k, 0:2],
                        rhs=rhs_for(s, k),
                        start=(k == 0), stop=(k == KC - 1),
                        tile_position=(0, POS[s]),
                        skip_group_check=True,
                    )
            # assemble lvec = [rowH; 1] and rvec = [1; colD+c] on the Scalar
            # engine: one 2-partition op each with per-partition scale/bias
            for b01 in range(2):
                b = 2 * p + b01
                lv, rv = lvecs[b % 4], rvecs[b % 4]
                lp, rp = POS[2 * b01], POS[2 * b01 + 1]
                nc.scalar.activation(
                    lv[0:2, :], v4[lp:lp + 2, :], ACT.Identity,
                    scale=cst_s[0:2, 0:1], bias=cst_s[0:2, 1:2],
                )
                nc.scalar.activation(
                    rv[0:2, :], v4[rp:rp + 2, :], ACT.Identity,
                    scale=cst_s[0:2, 2:3], bias=cst_s[0:2, 3:4],
                )

        def step1_pair(p):
            # G[i, y] = U1 @ H_b^T per batch; the PSUM->SBUF copy adds u2[i]
            # per partition, folding D.u2 into step 2.
            ht = ht_tiles[p]
            g2 = gpool.tile([P, KC, 2 * N], bf, name=f"g{p}", tag="g")
            if p == 0:
                # b01=0 jc-major: consumes (ujt[jc], ht[jc]) in DMA order
                banks = [
                    pspool.tile([P, N], f32, name=f"gA{ic}", tag="ps")
                    for ic in range(KC)
                ]
                for jc in range(KC):
                    for ic in range(KC):
                        nc.tensor.matmul(
                            banks[ic][:],
                            lhsT=ujt_s[:, jc, ic * P:(ic + 1) * P],
                            rhs=ht[:, jc, 0:N],
                            start=(jc == 0), stop=(jc == KC - 1),
                        )
                for ic in range(KC):
                    nc.vector.tensor_scalar_add(
                        g2[:, ic, 0:N], banks[ic][:], u2_s[:, ic:ic + 1]
                    )
                b01s = (1,)
            else:
                b01s = (0, 1)
            for ic in range(KC):
                for b01 in b01s:
                    g_ps = pspool.tile([P, N], f32, name=f"gps{p}_{ic}_{b01}", tag="ps")
                    for jc in range(KC):
                        nc.tensor.matmul(
                            g_ps[:],
                            lhsT=ujt_s[:, jc, ic * P:(ic + 1) * P],
                            rhs=ht[:, jc, b01 * N:(b01 + 1) * N],
                            start=(jc == 0), stop=(jc == KC - 1),
                        )
                    nc.vector.tensor_scalar_add(
                        g2[:, ic, b01 * N:(b01 + 1) * N], g_ps[:], u2_s[:, ic:ic + 1]
                    )
            return g2

        def step2(b, g2):
            b01 = b % 2
            dt = dt_tiles[b]
            lv, rv = lvecs[b % 4], rvecs[b % 4]
            # rank-2 terms (rowvec[x]*1 + 1*colvec[y]) first, batched: the
            # K=2 weight loads then never interrupt the uniform K=128 stream
            banks = []
            for xc in range(XC):
                s_ps = pspool.tile([P, N], f32, name=f"sps{b}_{xc}", tag="ps")
                banks.append(s_ps)
                nc.tensor.matmul(
                    s_ps[:],
                    lhsT=lv[:, xc * P:(xc + 1) * P],
                    rhs=rv[:, :],
                    start=True, stop=False,
                    skip_group_check=True,
                )
            for xc in range(XC):
                s_ps = banks[xc]
                for ic in range(KC):
                    nc.tensor.matmul(
                        s_ps[:],
                        lhsT=dt[:, ic, xc * P:(xc + 1) * P],
                        rhs=g2[:, ic, b01 * N:(b01 + 1) * N],
                        start=False, stop=(ic == KC - 1),
                        skip_group_check=True,
                    )
                o_t = opool.tile([P, N], f32, name=f"o{b}_{xc}", tag="o")
                if b == BPC - 1:
                    # last batch: alternate copy engines so the final two
                    # copy+DMA chains drain in parallel
                    if xc % 2 == 0:
                        nc.vector.tensor_copy(o_t[:], s_ps[:])
                        nc.sync.dma_start(out_h[b, xc * P:(xc + 1) * P, :], o_t[:])
                    else:
                        nc.scalar.activation(o_t[:], s_ps[:], ACT.Copy)
                        nc.scalar.dma_start(out_h[b, xc * P:(xc + 1) * P, :], o_t[:])
                else:
                    nc.scalar.activation(o_t[:], s_ps[:], ACT.Copy)
                    nc.scalar.dma_start(out_h[b, xc * P:(xc + 1) * P, :], o_t[:])

        g2 = step1_pair(0)
        vec_pair(0)
        step2(0, g2)
        step2(1, g2)
        for p in range(1, NP):
            vec_pair(p)
            g2 = step1_pair(p)
            step2(2 * p, g2)
            step2(2 * p + 1, g2)

    nc.finalize()
    return nc


def kernel(D, H, U, W, _trace=False):
    global LAST_RESULT
    _ensure_axon_ntff_hook()
    from concourse.bass_utils import run_bass_kernel_spmd

    D = np.asarray(D, dtype=np.float32)
    H = np.asarray(H, dtype=np.float32)
    U = np.asarray(U, dtype=np.float32)
    W = np.asarray(W, dtype=np.float32)

    # ---- host-side layout / dtype prep (no math beyond the W[d] scalar) ----
    # dtr[b, p, c, x] = D[b, x, c*128+p]  (D^T, chunked along the contraction dim)
    DT = D.transpose(0, 2, 1).astype(BF16)  # [B, DD, N]
    dtr = np.ascontiguousarray(DT.reshape(B, KC, P, N).transpose(0, 2, 1, 3))
    # htr[pair, jc, p, b01*N+n] = H[2*pair+b01, n, jc*128+p]; each (pair, jc)
    # chunk is one contiguous 512 KB block for big-line DMA
    HT = H.transpose(0, 2, 1).astype(BF16)  # [B, DD, N]
    htr = np.ascontiguousarray(
        HT.reshape(B // 2, 2, KC, P, N).transpose(0, 2, 3, 1, 4)
        .reshape(B // 2, KC, P, 2 * N)
    )
    # ujt[p, jc, i] = U[i, jc*128+p]
    U1T = U[:, :DD].T  # [j, i]
    ujt = np.ascontiguousarray(U1T.reshape(KC, P, DD).transpose(1, 0, 2)).astype(BF16)
    # vpr[p, c, :] = (wh, wd)[c*128+p] ; u2r[p, c] = u2[c*128+p] (fp32, folded into G)
    vp = np.stack([W[:DD], W[DD + 1:]], axis=1)  # [DD, 2]
    vpr = np.ascontiguousarray(vp.reshape(KC, P, 2).transpose(1, 0, 2)).astype(BF16)
    u2r = np.ascontiguousarray(U[:, DD].reshape(KC, P).T).astype(np.float32)
    c_const = float(W[DD])
    # (scale_l, bias_l, scale_r, bias_r) per partition row:
    # lvec row0 = rowH*1+0, row1 = junk*0+1 ; rvec row0 = junk*0+1, row1 = colD*1+c
    cst = np.array(
        [[1.0, 0.0, 0.0, 1.0], [0.0, 1.0, 1.0, c_const]], dtype=np.float32
    )

    nc = _build_bass(c_const)

    in_maps = []
    for c in range(NCORES):
        sl = slice(c * BPC, (c + 1) * BPC)
        slp = slice(c * NP, (c + 1) * NP)
        in_maps.append({
            "dtr": dtr[sl],
            "htr": htr[slp],
            "ujt": ujt,
            "vpr": vpr,
            "u2r": u2r,
            "cst": cst,
        })

    try:
        res = run_bass_kernel_spmd(
            nc, in_maps, core_ids=list(range(NCORES)), trace=_trace,
        )
    except Exception:
        # transient device errors (e.g. NRT_EXEC_UNIT_UNRECOVERABLE) usually
        # clear on retry
        res = run_bass_kernel_spmd(
            nc, in_maps, core_ids=list(range(NCORES)), trace=_trace,
        )
    LAST_RESULT = res

    out = np.concatenate([r["out"] for r in res.results], axis=0)
    return np.ascontiguousarray(out.astype(np.float32))


if __name__ == "__main__":
    rng = np.random.default_rng(0)
    D = rng.standard_normal((B, N, DD), dtype=np.float32)
    H = rng.standard_normal((B, N, DD), dtype=np.float32)
    U = (rng.standard_normal((DD, DD + 1)) * 0.02).astype(np.float32)
    W = (rng.standard_normal((2 * DD + 1,)) * 0.02).astype(np.float32)
    out = kernel(D=D, H=H, U=U, W=W)
    print(out.shape, out.dtype)
